# revision 14
# baseline (speedup 1.0000x reference)
"""Self-contained Trainium2 (Bass) kernel for the 2-layer GCN + MLP model.

Strategy (node-parallel, dst-sharded, two SPMD launches):
  * Host prep (index ops only): augment each node's incoming-edge list with a
    self-loop slot, CSR-sort by dst, shard nodes over 8 cores, bucket each
    core's nodes by slot count (deg+1), give every node a fixed (even) number
    of slots (bucket stride).  Slot streams are host-gathered into the slot
    layout as fp8e4m3: xg = x[src] (f-major) and wg = rsqrt(deg[src]+1) via a
    structure-derived palette (zero in padding slots).
  * Launch A (per core): SWDGE DMA casts fp8 -> fp16 during the load;
    y = xg*wg (fp16 tensor_tensor, DVE 2x); in-place halving-tree reduction
    over slots -> agg_f = z1_f/dinv (the self slot completes the aggregate);
    g2_ch = relu(agg@W1*dinv^2 + dinv*b1) = dinv*relu(z1@W1+b1); the layer-2
    input mix W2 commutes with the edge-sum, so A also pre-mixes
    u = g2@W2 (4ch -> 3ch) and stores u as fp8 [P, 3, SUM_M] planes
    (SWDGE cast on the store).
  * Host: gathers u[src] into the same slot layout (index gather only).
  * Launch B (per core): cast-load + halving-tree reduce of the 3-channel u
    slot stream (self slot supplies the own term) -> h2 = sigmoid(agg*dinv+b2)
    directly; then relu(.W3+b3) -> relu(.W4+b4) -> .W5+b5 with weights as
    immediates via tensor_scalar (4x) + tensor_tensor (2x) in fp16 and relu
    as tensor_scalar_max on the DVE (ACT does only sigmoid/sqrt).
  * Host: unpermute per-core outputs back to original node order.

The host only sorts, indexes, pads, concatenates and palette-gathers values
derived from the graph structure; all O(E)/O(N) feature math runs on device.
"""
import numpy as np

import concourse.bass as bass
from concourse.bacc import Bacc
import concourse.mybir as mybir
import concourse.tile as tile

NCORES = 8
N = 1_000_000
P = 128
F32 = mybir.dt.float32
F16 = mybir.dt.float16
F8 = mybir.dt.float8e4
F8NP = mybir.dt.np(mybir.dt.float8e4)
GS_F8 = True
AF = mybir.ActivationFunctionType
OP = mybir.AluOpType


# ----------------------------------------------------------------- host prep
def _choose_strides(max_used):
    max_even = int(max_used) + (int(max_used) & 1)
    ss = [s for s in (2, 4, 6, 8, 10, 12, 14, 16, 18, 20, 22, 24, 26, 28, 32,
                      36, 40, 48, 64, 96, 128, 192, 256, 384, 512)
          if s < max_even]
    ss.append(max_even)
    return ss


def _prep(x, edge_index, ncores=NCORES, n=N):
    npc = n // ncores
    src = np.asarray(edge_index[0]).astype(np.int64)
    dst = np.asarray(edge_index[1]).astype(np.int64)
    x = np.asarray(x, dtype=np.float32)
    deg_in = np.bincount(dst, minlength=n)          # in-degree (no self loop)
    used = deg_in + 1                               # slots incl. self
    strides = _choose_strides(max(int(used.max()), 2))
    strides_arr = np.asarray(strides)
    nb = len(strides)

    # augmented CSR: per dst node, slot 0 = self, slots 1.. = in-edge sources
    order = np.argsort(dst, kind="stable")
    src_s = src[order]
    aug_rowptr = np.zeros(n + 1, dtype=np.int64)
    np.cumsum(used, out=aug_rowptr[1:])
    aug_src = np.empty(n + int(len(src_s)), dtype=np.int64)
    aug_src[aug_rowptr[:-1]] = np.arange(n)
    m = np.ones(len(aug_src), dtype=bool)
    m[aug_rowptr[:-1]] = False
    aug_src[m] = src_s

    # rsqrt palette over deg+1 (structure-derived; fp16)
    pal = (1.0 / np.sqrt(np.arange(1, int(used.max()) + 1, dtype=np.float64)))
    pal = pal.astype(np.float32)
    w_node = pal[deg_in]                            # rsqrt(deg+1) per node

    bucket_of = np.searchsorted(strides_arr, used)

    m_b = np.zeros((ncores, nb), dtype=np.int64)
    node_lists = [[None] * nb for _ in range(ncores)]
    for c in range(ncores):
        lo, hi = c * npc, (c + 1) * npc
        nodes_c = np.arange(lo, hi)
        bk = bucket_of[lo:hi]
        for b in range(nb):
            nl = nodes_c[bk == b]
            node_lists[c][b] = nl
            m_b[c, b] = -(-len(nl) // P)
    m_pad = m_b.max(axis=0)
    SUM_M = -(-int(m_pad.sum()) // 32) * 32
    NPCP = P * SUM_M
    boff = np.concatenate([[0], np.cumsum(m_pad)]).astype(np.int64)
    SLOTS = int((m_pad * P * strides_arr).sum())

    def make_plan(target):
        cp = []
        for b in range(nb):
            s = strides[b]
            if m_pad[b] == 0:
                continue
            mc = max(32, -(-max(1, target // s) // 32) * 32)
            i = 0
            while i < m_pad[b]:
                take = int(min(mc, m_pad[b] - i))
                cp.append((b, s, int(i), take))
                i += take
        return cp
    chunk_plan = make_plan(8192)
    chunk_plan_B = make_plan(8192)

    storage = np.empty(n, dtype=np.int64)
    origin = np.full((ncores, NPCP), -1, dtype=np.int64)
    for c in range(ncores):
        for b in range(nb):
            nl, mb, off = node_lists[c][b], int(m_pad[b]), int(boff[b])
            if len(nl) == 0 or mb == 0:
                continue
            j = np.arange(len(nl))
            p, i = j // mb, j % mb
            sid = p * SUM_M + off + i
            storage[nl] = c * NPCP + sid
            origin[c, sid] = nl

    per_core = []
    for c in range(ncores):
        xg = np.zeros((SLOTS * 2,), dtype=np.float32)
        wgf = np.zeros((SLOTS,), dtype=np.float32)
        idxs = np.full((SLOTS,), ncores * NPCP, dtype=np.int64)  # pad row
        deg_own = np.ones((NPCP,), dtype=np.float32)
        sbase = 0
        for b in range(nb):
            s, mb = strides[b], int(m_pad[b])
            if mb == 0:
                continue
            nl = node_lists[c][b]
            if len(nl) > 0:
                j = np.arange(len(nl))
                p, i = j // mb, j % mb
                cnt = used[nl]
                node_rep = np.repeat(j, cnt)
                k_in = np.arange(len(node_rep)) - np.repeat(
                    np.concatenate([[0], np.cumsum(cnt)[:-1]]), cnt)
                e_pos = np.repeat(aug_rowptr[nl], cnt) + k_in
                slot = sbase + (p[node_rep] * mb + i[node_rep]) * s + k_in
                sv = aug_src[e_pos]
                slot_fm = sbase * 2 + (p[node_rep] * mb + i[node_rep]) * (2 * s) + k_in
                xg[slot_fm] = x[sv, 0]
                xg[slot_fm + s] = x[sv, 1]
                wgf[slot] = w_node[sv]
                idxs[slot] = storage[sv]
            sbase += P * mb * s
        assert sbase == SLOTS

        valid = origin[c] >= 0
        ov = origin[c][valid]
        deg_own[valid] = (deg_in[ov] + 1).astype(np.float32)
        per_core.append(dict(xg=xg.astype(F8NP), wg=wgf.astype(F8NP),
                             idxs=idxs, deg_own=deg_own))

    meta = dict(strides=strides, m_pad=m_pad, SUM_M=SUM_M, NPCP=NPCP,
                boff=boff, SLOTS=SLOTS, chunk_plan=chunk_plan,
                chunk_plan_B=chunk_plan_B, origin=origin,
                ncores=ncores, n=n)
    return per_core, meta


# ------------------------------------------------------------- device common
def _sbases(meta):
    sbases = {}
    sb = 0
    for b, s in enumerate(meta["strides"]):
        sbases[b] = sb
        sb += P * int(meta["m_pad"][b]) * s
    return sbases


def _tree_reduce(nc, t4, s, final_out):
    """In-place halving-tree sum over the last axis of t4 [P, mc, nch, s]
    (s even, >= 2); the final level writes per-channel totals to
    final_out [P, mc, nch] in a single op."""
    cur = s
    while cur > 2:
        h = cur // 2
        nc.vector.tensor_tensor(out=t4[:, :, :, 0:h], in0=t4[:, :, :, 0:h],
                                in1=t4[:, :, :, cur - h:cur], op=OP.add)
        cur -= h
    nc.vector.tensor_tensor(out=final_out, in0=t4[:, :, :, 0],
                            in1=t4[:, :, :, 1], op=OP.add)


def _dinv_chain(nc, res, deg_own, SUM_M):
    dinv = res.tile([P, SUM_M], F32, tag="dinv")
    nc.sync.dma_start(out=dinv[:],
                      in_=deg_own[:].rearrange("(p j) -> p j", p=P))
    nc.vector.reciprocal(out=dinv[:], in_=dinv[:])
    nc.scalar.activation(out=dinv[:], in_=dinv[:], func=AF.Sqrt)
    return dinv


# --------------------------------------------------------- device build: A
def _build_A(meta, W1b, W2, reps=1):
    SUM_M, SLOTS = meta["SUM_M"], meta["SLOTS"]
    m_pad, boff = meta["m_pad"], meta["boff"]
    plan = meta["chunk_plan"]
    sbases = _sbases(meta)

    nc = Bacc(num_devices=meta["ncores"])
    xg = nc.declare_dram_parameter("xg", [SLOTS * 2], F8, isOutput=False)
    wg = nc.declare_dram_parameter("wg", [SLOTS], F8, isOutput=False)
    deg_own = nc.declare_dram_parameter("deg_own", [meta["NPCP"]], F32,
                                        isOutput=False)
    g2out = nc.declare_dram_parameter("g2out", [P, 3, SUM_M], F8,
                                      isOutput=True)

    with tile.TileContext(nc) as tc:
        with tc.tile_pool(name="res", bufs=1) as res:
            for _ in range(reps):
                dinv = _dinv_chain(nc, res, deg_own, SUM_M)
                dsqb = res.tile([P, SUM_M], F16, tag="dsqb")
                nc.vector.tensor_tensor(out=dsqb[:], in0=dinv[:], in1=dinv[:],
                                        op=OP.mult)
                dinvb = res.tile([P, SUM_M], F16, tag="dinvb")
                nc.scalar.copy(out=dinvb[:], in_=dinv[:])

                agg2t = res.tile([P, SUM_M, 2], F16, tag="agg2t")
                nc.vector.memset(agg2t[:], 0.0)

                with tc.tile_pool(name="l1", bufs=2) as st:
                    for (b, s, i0, mc) in plan:
                        mb = int(m_pad[b])
                        xv = xg[2 * sbases[b]:2 * (sbases[b] + P * mb * s)] \
                            .rearrange("(p i fk) -> p i fk", p=P, i=mb)[:, i0:i0 + mc, :]
                        wv = wg[sbases[b]:sbases[b] + P * mb * s] \
                            .rearrange("(p i k) -> p i k", p=P, i=mb)[:, i0:i0 + mc, :]
                        xt = st.tile([P, mc, 2 * s], F16, tag="xg")
                        wt = st.tile([P, mc, s], F16, tag="wg")
                        nc.gpsimd.dma_start(out=xt[:], in_=xv)
                        nc.gpsimd.dma_start(out=wt[:], in_=wv)
                        for f in range(2):
                            sl = xt[:, :, f * s:(f + 1) * s]
                            nc.vector.tensor_tensor(out=sl, in0=sl, in1=wt[:],
                                                    op=OP.mult)
                        j0 = int(boff[b]) + i0
                        xt4 = xt[:].rearrange("p i (f k) -> p i f k", k=s)
                        _tree_reduce(nc, xt4, s, agg2t[:, j0:j0 + mc, :])

                # g2_ch = relu(agg@W1*dinv^2 + dinv*b1); u_o = sum_ch g2_ch*W2[ch,o]
                af3 = [res.tile([P, SUM_M], F16, tag=f"af3{f}",
                                name=f"af3{f}") for f in range(2)]
                for f in range(2):
                    nc.vector.tensor_tensor(out=af3[f][:], in0=agg2t[:, :, f],
                                            in1=dsqb[:], op=OP.mult)
                g2r = [res.tile([P, SUM_M], F16, tag=f"g2r{o}",
                                name=f"g2r{o}") for o in range(4)]
                ti = res.tile([P, SUM_M], F16, tag="ti")
                for o in range(4):
                    go = g2r[o]
                    nc.vector.tensor_scalar(out=go[:], in0=af3[0][:],
                                            scalar1=float(W1b[0, o]),
                                            scalar2=None, op0=OP.mult)
                    nc.vector.tensor_scalar(out=ti[:], in0=af3[1][:],
                                            scalar1=float(W1b[1, o]),
                                            scalar2=None, op0=OP.mult)
                    nc.vector.tensor_tensor(out=go[:], in0=go[:], in1=ti[:],
                                            op=OP.add)
                    nc.vector.tensor_scalar(out=ti[:], in0=dinvb[:],
                                            scalar1=float(W1b[2, o]),
                                            scalar2=None, op0=OP.mult)
                    nc.vector.tensor_tensor(out=go[:], in0=go[:], in1=ti[:],
                                            op=OP.add)
                    nc.vector.tensor_scalar_max(out=go[:], in0=go[:],
                                                scalar1=0.0)
                up = res.tile([P, 3, SUM_M], F16, tag="up")
                for o in range(3):
                    uo = up[:, o, :]
                    nc.vector.tensor_scalar(out=uo, in0=g2r[0][:],
                                            scalar1=float(W2[0, o]),
                                            scalar2=None, op0=OP.mult)
                    for ch in range(1, 4):
                        nc.vector.tensor_scalar(out=ti[:], in0=g2r[ch][:],
                                                scalar1=float(W2[ch, o]),
                                                scalar2=None, op0=OP.mult)
                        nc.vector.tensor_tensor(out=uo, in0=uo, in1=ti[:],
                                                op=OP.add)
                nc.gpsimd.dma_start(out=g2out[:], in_=up[:])
    return nc


# --------------------------------------------------------- device build: B
def _build_B(meta, weights, reps=1):
    SUM_M, SLOTS = meta["SUM_M"], meta["SLOTS"]
    m_pad, boff = meta["m_pad"], meta["boff"]
    plan = meta["chunk_plan_B"]
    sbases = _sbases(meta)
    W2, b2 = weights["W2"], weights["b2"]
    W3, b3 = weights["W3"], weights["b3"]
    W4, b4 = weights["W4"], weights["b4"]
    W5, b5 = weights["W5"], weights["b5"]

    nc = Bacc(num_devices=meta["ncores"])
    gs = nc.declare_dram_parameter("gs", [SLOTS * 3],
                                   F8 if GS_F8 else F16, isOutput=False)
    deg_own = nc.declare_dram_parameter("deg_own", [meta["NPCP"]], F32,
                                        isOutput=False)
    out = nc.declare_dram_parameter("out", [P, SUM_M], F32, isOutput=True)

    with tile.TileContext(nc) as tc:
        with tc.tile_pool(name="res", bufs=1) as res:
            for _ in range(reps):
                dinv = _dinv_chain(nc, res, deg_own, SUM_M)
                dinvb = res.tile([P, SUM_M], F16, tag="dinvb")
                nc.scalar.copy(out=dinvb[:], in_=dinv[:])

                agg3 = res.tile([P, SUM_M, 3], F16, tag="agg3")
                nc.vector.memset(agg3[:], 0.0)

                with tc.tile_pool(name="l2", bufs=2) as st:
                    for (b, s, i0, mc) in plan:
                        mb = int(m_pad[b])
                        gv = gs[3 * sbases[b]:3 * (sbases[b] + P * mb * s)] \
                            .rearrange("(p i fk) -> p i fk", p=P, i=mb)[:, i0:i0 + mc, :]
                        gt = st.tile([P, mc, 3 * s], F16, tag="gath")
                        if GS_F8:
                            nc.gpsimd.dma_start(out=gt[:], in_=gv)
                        else:
                            nc.sync.dma_start(out=gt[:], in_=gv)
                        j0 = int(boff[b]) + i0
                        gt4 = gt[:].rearrange("p i (f k) -> p i f k", k=s)
                        _tree_reduce(nc, gt4, s, agg3[:, j0:j0 + mc, :])

                # h2_o = sigmoid(aggu_o*dinv + b2_o)   (W2 pre-mixed in launch A)
                h2 = []
                for o in range(3):
                    h = res.tile([P, SUM_M], F16, tag=f"h2_{o}",
                                 name=f"h2_{o}")
                    bt = res.tile([P, 1], F32, tag=f"h2b{o}", name=f"h2b{o}")
                    nc.vector.memset(bt[:], float(b2[o]))
                    nc.vector.tensor_tensor(out=h[:], in0=agg3[:, :, o],
                                            in1=dinvb[:], op=OP.mult)
                    nc.scalar.activation(out=h[:], in_=h[:], func=AF.Sigmoid,
                                         bias=bt[:])
                    h2.append(h)

                ti = res.tile([P, SUM_M], F16, tag="ti")

                def dense(ins_, Wm, bias, act, tagp, och, out_f32=None):
                    outs_ = []
                    for o in range(och):
                        acc = (out_f32 if out_f32 is not None else
                               res.tile([P, SUM_M], F16, tag=f"{tagp}{o}",
                                        name=f"{tagp}{o}"))
                        nc.vector.tensor_scalar(out=acc[:], in0=ins_[0][:],
                                                scalar1=float(Wm[0, o]),
                                                scalar2=float(bias[o]),
                                                op0=OP.mult, op1=OP.add)
                        for i in range(1, len(ins_)):
                            nc.vector.tensor_scalar(out=ti[:], in0=ins_[i][:],
                                                    scalar1=float(Wm[i, o]),
                                                    scalar2=None, op0=OP.mult)
                            nc.vector.tensor_tensor(out=acc[:], in0=acc[:],
                                                    in1=ti[:], op=OP.add)
                        if act == "sigmoid":
                            nc.scalar.activation(out=acc[:], in_=acc[:],
                                                 func=AF.Sigmoid)
                        elif act == "relu":
                            nc.vector.tensor_scalar_max(out=acc[:], in0=acc[:],
                                                        scalar1=0.0)
                        outs_.append(acc)
                    return outs_

                h = dense(h2, W3, b3, "relu", "h3_", 4)
                h = dense(h, W4, b4, "relu", "h4_", 3)
                outp = res.tile([P, SUM_M], F32, tag="outp")
                dense(h, W5, b5, None, "h5_", 1, out_f32=outp)
                nc.sync.dma_start(out=out[:], in_=outp[:])
    return nc


# ------------------------------------------------------------------ driver
def _run_spmd(nc, in_maps, ncores):
    from concourse.bass_utils import run_bass_kernel_spmd
    if not nc.is_finalized():
        nc.finalize()
    return run_bass_kernel_spmd(nc, in_maps, core_ids=list(range(ncores)))


def host_gather_g2(meta, per_core, g2_slices):
    """g2_slices[c]: [P, 3, SUM_M] fp8 u-planes from launch A. Returns
    per-core slot streams [SLOTS*3] in ch-major slot layout (index gather)."""
    NC, NPCP = meta["ncores"], meta["NPCP"]
    strides, m_pad = meta["strides"], meta["m_pad"]
    g2_nodes = [np.asarray(g2_slices[c]).transpose(0, 2, 1).reshape(NPCP, 3)
                for c in range(NC)]
    g2_full = np.concatenate(
        g2_nodes + [np.zeros((1, 3), g2_nodes[0].dtype)], axis=0)
    out = []
    for c in range(NC):
        g = g2_full[per_core[c]["idxs"]]          # [SLOTS, 3] slot-major
        fm = np.empty((meta["SLOTS"] * 3,), dtype=g2_full.dtype)
        sbase = 0
        for b, s in enumerate(strides):
            mb = int(m_pad[b])
            if mb == 0:
                continue
            nseg = P * mb * s
            seg = g[sbase:sbase + nseg].reshape(P * mb, s, 3)
            fm[sbase * 3:(sbase + nseg) * 3] = \
                seg.transpose(0, 2, 1).reshape(-1)
            sbase += nseg
        out.append(fm if GS_F8 else fm.astype(np.float16))
    return out


def kernel(x, edge_index, W1, b1, W2, b2, W3, b3, W4, b4, W5, b5):
    x = np.asarray(x, dtype=np.float32)
    per_core, meta = _prep(x, edge_index)
    W1b = np.concatenate([np.asarray(W1), np.asarray(b1)[None, :]], axis=0)
    weights = dict(W2=np.asarray(W2), b2=np.asarray(b2),
                   W3=np.asarray(W3), b3=np.asarray(b3),
                   W4=np.asarray(W4), b4=np.asarray(b4),
                   W5=np.asarray(W5), b5=np.asarray(b5))
    NC = meta["ncores"]

    ncA = _build_A(meta, W1b, np.asarray(W2))
    resA = _run_spmd(ncA, [{k: d[k] for k in ("xg", "wg", "deg_own")}
                           for d in per_core], NC)
    g2_slices = [resA.results[c]["g2out"] for c in range(NC)]

    gs = host_gather_g2(meta, per_core, g2_slices)
    ncB = _build_B(meta, weights)
    resB = _run_spmd(ncB, [dict(gs=gs[c], deg_own=per_core[c]["deg_own"])
                           for c in range(NC)], NC)

    full = np.zeros(meta["n"], dtype=np.float32)
    for c in range(NC):
        o = np.asarray(resB.results[c]["out"]).reshape(-1)
        org = meta["origin"][c]
        valid = org >= 0
        full[org[valid]] = o[valid]
    return full


# revision 15
# speedup vs baseline: 2.2178x; 2.2178x over previous
"""Self-contained Trainium2 (Bass) kernel for the 2-layer GCN + MLP model.

Strategy (node-parallel, dst-sharded, two SPMD launches):
  * Host prep (index ops only): augment each node's incoming-edge list with a
    self-loop slot, CSR-sort by dst, shard nodes over 8 cores, bucket each
    core's nodes by slot count (deg+1), give every node a fixed (even) number
    of slots (bucket stride).  Slot streams are host-gathered into the slot
    layout as fp8e4m3: xg = x[src] (f-major) and wg = rsqrt(deg[src]+1) via a
    structure-derived palette (zero in padding slots).
  * Launch A (per core): SWDGE DMA casts fp8 -> fp16 during the load;
    y = xg*wg (fp16 tensor_tensor, DVE 2x); in-place halving-tree reduction
    over slots -> agg_f = z1_f/dinv (the self slot completes the aggregate);
    g2_ch = relu(agg@W1*dinv^2 + dinv*b1) = dinv*relu(z1@W1+b1); the layer-2
    input mix W2 commutes with the edge-sum, so A also pre-mixes
    u = g2@W2 (4ch -> 3ch) and stores u as fp8 [P, 3, SUM_M] planes
    (SWDGE cast on the store).
  * Host: gathers u[src] into the same slot layout (index gather only).
  * Launch B (per core): cast-load + halving-tree reduce of the 3-channel u
    slot stream (self slot supplies the own term) -> h2 = sigmoid(agg*dinv+b2)
    directly; then relu(.W3+b3) -> relu(.W4+b4) -> .W5+b5 with weights as
    immediates via tensor_scalar (4x) + tensor_tensor (2x) in fp16 and relu
    as tensor_scalar_max on the DVE (ACT does only sigmoid/sqrt).
  * Host: unpermute per-core outputs back to original node order.

The host only sorts, indexes, pads, concatenates and palette-gathers values
derived from the graph structure; all O(E)/O(N) feature math runs on device.
"""
import numpy as np

import concourse.bass as bass
from concourse.bacc import Bacc
import concourse.mybir as mybir
import concourse.tile as tile

NCORES = 8
N = 1_000_000
P = 128
F32 = mybir.dt.float32
F16 = mybir.dt.float16
F8 = mybir.dt.float8e4
F8NP = mybir.dt.np(mybir.dt.float8e4)
GS_F8 = True
AF = mybir.ActivationFunctionType
OP = mybir.AluOpType


# ----------------------------------------------------------------- host prep
def _choose_strides(max_used):
    max_even = int(max_used) + (int(max_used) & 1)
    ss = [s for s in (2, 4, 6, 8, 10, 12, 14, 16, 18, 20, 22, 24, 26, 28, 32,
                      36, 40, 48, 64, 96, 128, 192, 256, 384, 512)
          if s < max_even]
    ss.append(max_even)
    return ss


def _prep(x, edge_index, ncores=NCORES, n=N):
    npc = n // ncores
    src = np.asarray(edge_index[0]).astype(np.int64)
    dst = np.asarray(edge_index[1]).astype(np.int64)
    x = np.asarray(x, dtype=np.float32)
    deg_in = np.bincount(dst, minlength=n)          # in-degree (no self loop)
    used = deg_in + 1                               # slots incl. self
    strides = _choose_strides(max(int(used.max()), 2))
    strides_arr = np.asarray(strides)
    nb = len(strides)

    # augmented CSR: per dst node, slot 0 = self, slots 1.. = in-edge sources
    order = np.argsort(dst, kind="stable")
    src_s = src[order]
    aug_rowptr = np.zeros(n + 1, dtype=np.int64)
    np.cumsum(used, out=aug_rowptr[1:])
    aug_src = np.empty(n + int(len(src_s)), dtype=np.int64)
    aug_src[aug_rowptr[:-1]] = np.arange(n)
    m = np.ones(len(aug_src), dtype=bool)
    m[aug_rowptr[:-1]] = False
    aug_src[m] = src_s

    # rsqrt palette over deg+1 (structure-derived; fp16)
    pal = (1.0 / np.sqrt(np.arange(1, int(used.max()) + 1, dtype=np.float64)))
    pal = pal.astype(np.float32)
    w_node = pal[deg_in]                            # rsqrt(deg+1) per node

    bucket_of = np.searchsorted(strides_arr, used)

    m_b = np.zeros((ncores, nb), dtype=np.int64)
    node_lists = [[None] * nb for _ in range(ncores)]
    for c in range(ncores):
        lo, hi = c * npc, (c + 1) * npc
        nodes_c = np.arange(lo, hi)
        bk = bucket_of[lo:hi]
        for b in range(nb):
            nl = nodes_c[bk == b]
            node_lists[c][b] = nl
            m_b[c, b] = -(-len(nl) // P)
    m_pad = m_b.max(axis=0)
    SUM_M = -(-int(m_pad.sum()) // 32) * 32
    NPCP = P * SUM_M
    boff = np.concatenate([[0], np.cumsum(m_pad)]).astype(np.int64)
    SLOTS = int((m_pad * P * strides_arr).sum())

    def make_plan(target):
        cp = []
        for b in range(nb):
            s = strides[b]
            if m_pad[b] == 0:
                continue
            mc = max(32, -(-max(1, target // s) // 32) * 32)
            i = 0
            while i < m_pad[b]:
                take = int(min(mc, m_pad[b] - i))
                cp.append((b, s, int(i), take))
                i += take
        return cp
    chunk_plan = make_plan(8192)
    chunk_plan_B = make_plan(8192)

    storage = np.empty(n, dtype=np.int64)
    origin = np.full((ncores, NPCP), -1, dtype=np.int64)
    for c in range(ncores):
        for b in range(nb):
            nl, mb, off = node_lists[c][b], int(m_pad[b]), int(boff[b])
            if len(nl) == 0 or mb == 0:
                continue
            j = np.arange(len(nl))
            p, i = j // mb, j % mb
            sid = p * SUM_M + off + i
            storage[nl] = c * NPCP + sid
            origin[c, sid] = nl

    per_core = []
    for c in range(ncores):
        xg = np.zeros((SLOTS * 2,), dtype=np.float32)
        wgf = np.zeros((SLOTS,), dtype=np.float32)
        idxs = np.full((SLOTS,), ncores * NPCP, dtype=np.int64)  # pad row
        dinv_own = np.ones((NPCP,), dtype=np.float16)
        sbase = 0
        for b in range(nb):
            s, mb = strides[b], int(m_pad[b])
            if mb == 0:
                continue
            nl = node_lists[c][b]
            if len(nl) > 0:
                j = np.arange(len(nl))
                p, i = j // mb, j % mb
                cnt = used[nl]
                node_rep = np.repeat(j, cnt)
                k_in = np.arange(len(node_rep)) - np.repeat(
                    np.concatenate([[0], np.cumsum(cnt)[:-1]]), cnt)
                e_pos = np.repeat(aug_rowptr[nl], cnt) + k_in
                slot = sbase + (p[node_rep] * mb + i[node_rep]) * s + k_in
                sv = aug_src[e_pos]
                slot_fm = sbase * 2 + (p[node_rep] * mb + i[node_rep]) * (2 * s) + k_in
                xg[slot_fm] = x[sv, 0]
                xg[slot_fm + s] = x[sv, 1]
                wgf[slot] = w_node[sv]
                idxs[slot] = storage[sv]
            sbase += P * mb * s
        assert sbase == SLOTS

        valid = origin[c] >= 0
        ov = origin[c][valid]
        dinv_own[valid] = w_node[ov].astype(np.float16)
        per_core.append(dict(xg=xg.astype(F8NP), wg=wgf.astype(F8NP),
                             idxs=idxs, dinv_own=dinv_own))

    meta = dict(strides=strides, m_pad=m_pad, SUM_M=SUM_M, NPCP=NPCP,
                boff=boff, SLOTS=SLOTS, chunk_plan=chunk_plan,
                chunk_plan_B=chunk_plan_B, origin=origin,
                ncores=ncores, n=n)
    return per_core, meta


# ------------------------------------------------------------- device common
def _sbases(meta):
    sbases = {}
    sb = 0
    for b, s in enumerate(meta["strides"]):
        sbases[b] = sb
        sb += P * int(meta["m_pad"][b]) * s
    return sbases


def _tree_reduce(nc, t4, s, final_out):
    """In-place halving-tree sum over the last axis of t4 [P, mc, nch, s]
    (s even, >= 2); the final level writes per-channel totals to
    final_out [P, mc, nch] in a single op."""
    cur = s
    while cur > 2:
        h = cur // 2
        nc.vector.tensor_tensor(out=t4[:, :, :, 0:h], in0=t4[:, :, :, 0:h],
                                in1=t4[:, :, :, cur - h:cur], op=OP.add)
        cur -= h
    nc.vector.tensor_tensor(out=final_out, in0=t4[:, :, :, 0],
                            in1=t4[:, :, :, 1], op=OP.add)


def _dinv_chain(nc, res, dinv_own, SUM_M):
    dinvb = res.tile([P, SUM_M], F16, tag="dinvb")
    nc.sync.dma_start(out=dinvb[:],
                      in_=dinv_own[:].rearrange("(p j) -> p j", p=P))
    return dinvb


# --------------------------------------------------------- device build: A
def _build_A(meta, W1b, W2, reps=1):
    SUM_M, SLOTS = meta["SUM_M"], meta["SLOTS"]
    m_pad, boff = meta["m_pad"], meta["boff"]
    plan = meta["chunk_plan"]
    sbases = _sbases(meta)

    nc = Bacc(num_devices=meta["ncores"])
    xg = nc.declare_dram_parameter("xg", [SLOTS * 2], F8, isOutput=False)
    wg = nc.declare_dram_parameter("wg", [SLOTS], F8, isOutput=False)
    dinv_own = nc.declare_dram_parameter("dinv_own", [meta["NPCP"]], F16,
                                         isOutput=False)
    g2out = nc.declare_dram_parameter("g2out", [P, 3, SUM_M], F8,
                                      isOutput=True)

    with tile.TileContext(nc) as tc:
        with tc.tile_pool(name="res", bufs=1) as res:
            for _ in range(reps):
                dinvb = _dinv_chain(nc, res, dinv_own, SUM_M)
                dsqb = res.tile([P, SUM_M], F16, tag="dsqb")
                nc.vector.tensor_tensor(out=dsqb[:], in0=dinvb[:],
                                        in1=dinvb[:], op=OP.mult)

                agg2t = res.tile([P, SUM_M, 2], F16, tag="agg2t")
                nc.vector.memset(agg2t[:], 0.0)

                with tc.tile_pool(name="l1", bufs=3) as st:
                    for (b, s, i0, mc) in plan:
                        mb = int(m_pad[b])
                        xv = xg[2 * sbases[b]:2 * (sbases[b] + P * mb * s)] \
                            .rearrange("(p i fk) -> p i fk", p=P, i=mb)[:, i0:i0 + mc, :]
                        wv = wg[sbases[b]:sbases[b] + P * mb * s] \
                            .rearrange("(p i k) -> p i k", p=P, i=mb)[:, i0:i0 + mc, :]
                        xt = st.tile([P, mc, 2 * s], F16, tag="xg")
                        wt = st.tile([P, mc, s], F16, tag="wg")
                        nc.gpsimd.dma_start(out=xt[:], in_=xv)
                        nc.gpsimd.dma_start(out=wt[:], in_=wv)
                        for f in range(2):
                            sl = xt[:, :, f * s:(f + 1) * s]
                            nc.vector.tensor_tensor(out=sl, in0=sl, in1=wt[:],
                                                    op=OP.mult)
                        j0 = int(boff[b]) + i0
                        xt4 = xt[:].rearrange("p i (f k) -> p i f k", k=s)
                        _tree_reduce(nc, xt4, s, agg2t[:, j0:j0 + mc, :])

                # g2_ch = relu(agg@W1*dinv^2 + dinv*b1); u_o = sum_ch g2_ch*W2[ch,o]
                af3 = [res.tile([P, SUM_M], F16, tag=f"af3{f}",
                                name=f"af3{f}") for f in range(2)]
                for f in range(2):
                    nc.vector.tensor_tensor(out=af3[f][:], in0=agg2t[:, :, f],
                                            in1=dsqb[:], op=OP.mult)
                g2r = [res.tile([P, SUM_M], F16, tag=f"g2r{o}",
                                name=f"g2r{o}") for o in range(4)]
                ti = res.tile([P, SUM_M], F16, tag="ti")
                for o in range(4):
                    go = g2r[o]
                    nc.vector.tensor_scalar(out=go[:], in0=af3[0][:],
                                            scalar1=float(W1b[0, o]),
                                            scalar2=None, op0=OP.mult)
                    nc.vector.tensor_scalar(out=ti[:], in0=af3[1][:],
                                            scalar1=float(W1b[1, o]),
                                            scalar2=None, op0=OP.mult)
                    nc.vector.tensor_tensor(out=go[:], in0=go[:], in1=ti[:],
                                            op=OP.add)
                    nc.vector.tensor_scalar(out=ti[:], in0=dinvb[:],
                                            scalar1=float(W1b[2, o]),
                                            scalar2=None, op0=OP.mult)
                    nc.vector.tensor_tensor(out=go[:], in0=go[:], in1=ti[:],
                                            op=OP.add)
                    nc.vector.tensor_scalar_max(out=go[:], in0=go[:],
                                                scalar1=0.0)
                up = res.tile([P, 3, SUM_M], F16, tag="up")
                for o in range(3):
                    uo = up[:, o, :]
                    nc.vector.tensor_scalar(out=uo, in0=g2r[0][:],
                                            scalar1=float(W2[0, o]),
                                            scalar2=None, op0=OP.mult)
                    for ch in range(1, 4):
                        nc.vector.tensor_scalar(out=ti[:], in0=g2r[ch][:],
                                                scalar1=float(W2[ch, o]),
                                                scalar2=None, op0=OP.mult)
                        nc.vector.tensor_tensor(out=uo, in0=uo, in1=ti[:],
                                                op=OP.add)
                nc.gpsimd.dma_start(out=g2out[:], in_=up[:])
    return nc


# --------------------------------------------------------- device build: B
def _build_B(meta, weights, reps=1):
    SUM_M, SLOTS = meta["SUM_M"], meta["SLOTS"]
    m_pad, boff = meta["m_pad"], meta["boff"]
    plan = meta["chunk_plan_B"]
    sbases = _sbases(meta)
    W2, b2 = weights["W2"], weights["b2"]
    W3, b3 = weights["W3"], weights["b3"]
    W4, b4 = weights["W4"], weights["b4"]
    W5, b5 = weights["W5"], weights["b5"]

    nc = Bacc(num_devices=meta["ncores"])
    gs = nc.declare_dram_parameter("gs", [SLOTS * 3],
                                   F8 if GS_F8 else F16, isOutput=False)
    dinv_own = nc.declare_dram_parameter("dinv_own", [meta["NPCP"]], F16,
                                         isOutput=False)
    out = nc.declare_dram_parameter("out", [P, SUM_M], F32, isOutput=True)

    with tile.TileContext(nc) as tc:
        with tc.tile_pool(name="res", bufs=1) as res:
            for _ in range(reps):
                dinvb = _dinv_chain(nc, res, dinv_own, SUM_M)

                agg3 = res.tile([P, SUM_M, 3], F16, tag="agg3")
                nc.vector.memset(agg3[:], 0.0)

                with tc.tile_pool(name="l2", bufs=3) as st:
                    for (b, s, i0, mc) in plan:
                        mb = int(m_pad[b])
                        gv = gs[3 * sbases[b]:3 * (sbases[b] + P * mb * s)] \
                            .rearrange("(p i fk) -> p i fk", p=P, i=mb)[:, i0:i0 + mc, :]
                        gt = st.tile([P, mc, 3 * s], F16, tag="gath")
                        if GS_F8:
                            nc.gpsimd.dma_start(out=gt[:], in_=gv)
                        else:
                            nc.sync.dma_start(out=gt[:], in_=gv)
                        j0 = int(boff[b]) + i0
                        gt4 = gt[:].rearrange("p i (f k) -> p i f k", k=s)
                        _tree_reduce(nc, gt4, s, agg3[:, j0:j0 + mc, :])

                # h2_o = sigmoid(aggu_o*dinv + b2_o)   (W2 pre-mixed in launch A)
                h2 = []
                for o in range(3):
                    h = res.tile([P, SUM_M], F16, tag=f"h2_{o}",
                                 name=f"h2_{o}")
                    bt = res.tile([P, 1], F32, tag=f"h2b{o}", name=f"h2b{o}")
                    nc.vector.memset(bt[:], float(b2[o]))
                    nc.vector.tensor_tensor(out=h[:], in0=agg3[:, :, o],
                                            in1=dinvb[:], op=OP.mult)
                    nc.scalar.activation(out=h[:], in_=h[:], func=AF.Sigmoid,
                                         bias=bt[:])
                    h2.append(h)

                ti = res.tile([P, SUM_M], F16, tag="ti")

                def dense(ins_, Wm, bias, act, tagp, och, out_f32=None):
                    outs_ = []
                    for o in range(och):
                        acc = (out_f32 if out_f32 is not None else
                               res.tile([P, SUM_M], F16, tag=f"{tagp}{o}",
                                        name=f"{tagp}{o}"))
                        nc.vector.tensor_scalar(out=acc[:], in0=ins_[0][:],
                                                scalar1=float(Wm[0, o]),
                                                scalar2=float(bias[o]),
                                                op0=OP.mult, op1=OP.add)
                        for i in range(1, len(ins_)):
                            nc.vector.tensor_scalar(out=ti[:], in0=ins_[i][:],
                                                    scalar1=float(Wm[i, o]),
                                                    scalar2=None, op0=OP.mult)
                            nc.vector.tensor_tensor(out=acc[:], in0=acc[:],
                                                    in1=ti[:], op=OP.add)
                        if act == "sigmoid":
                            nc.scalar.activation(out=acc[:], in_=acc[:],
                                                 func=AF.Sigmoid)
                        elif act == "relu":
                            nc.vector.tensor_scalar_max(out=acc[:], in0=acc[:],
                                                        scalar1=0.0)
                        outs_.append(acc)
                    return outs_

                h = dense(h2, W3, b3, "relu", "h3_", 4)
                h = dense(h, W4, b4, "relu", "h4_", 3)
                outp = res.tile([P, SUM_M], F32, tag="outp")
                dense(h, W5, b5, None, "h5_", 1, out_f32=outp)
                nc.sync.dma_start(out=out[:], in_=outp[:])
    return nc


# ------------------------------------------------------------------ driver
def _run_spmd(nc, in_maps, ncores):
    from concourse.bass_utils import run_bass_kernel_spmd
    if not nc.is_finalized():
        nc.finalize()
    return run_bass_kernel_spmd(nc, in_maps, core_ids=list(range(ncores)))


def host_gather_g2(meta, per_core, g2_slices):
    """g2_slices[c]: [P, 3, SUM_M] fp8 u-planes from launch A. Returns
    per-core slot streams [SLOTS*3] in ch-major slot layout (index gather)."""
    NC, NPCP = meta["ncores"], meta["NPCP"]
    strides, m_pad = meta["strides"], meta["m_pad"]
    g2_nodes = [np.asarray(g2_slices[c]).transpose(0, 2, 1).reshape(NPCP, 3)
                for c in range(NC)]
    g2_full = np.concatenate(
        g2_nodes + [np.zeros((1, 3), g2_nodes[0].dtype)], axis=0)
    out = []
    for c in range(NC):
        g = g2_full[per_core[c]["idxs"]]          # [SLOTS, 3] slot-major
        fm = np.empty((meta["SLOTS"] * 3,), dtype=g2_full.dtype)
        sbase = 0
        for b, s in enumerate(strides):
            mb = int(m_pad[b])
            if mb == 0:
                continue
            nseg = P * mb * s
            seg = g[sbase:sbase + nseg].reshape(P * mb, s, 3)
            fm[sbase * 3:(sbase + nseg) * 3] = \
                seg.transpose(0, 2, 1).reshape(-1)
            sbase += nseg
        out.append(fm if GS_F8 else fm.astype(np.float16))
    return out


def kernel(x, edge_index, W1, b1, W2, b2, W3, b3, W4, b4, W5, b5):
    x = np.asarray(x, dtype=np.float32)
    per_core, meta = _prep(x, edge_index)
    W1b = np.concatenate([np.asarray(W1), np.asarray(b1)[None, :]], axis=0)
    weights = dict(W2=np.asarray(W2), b2=np.asarray(b2),
                   W3=np.asarray(W3), b3=np.asarray(b3),
                   W4=np.asarray(W4), b4=np.asarray(b4),
                   W5=np.asarray(W5), b5=np.asarray(b5))
    NC = meta["ncores"]

    ncA = _build_A(meta, W1b, np.asarray(W2))
    resA = _run_spmd(ncA, [{k: d[k] for k in ("xg", "wg", "dinv_own")}
                           for d in per_core], NC)
    g2_slices = [resA.results[c]["g2out"] for c in range(NC)]

    gs = host_gather_g2(meta, per_core, g2_slices)
    ncB = _build_B(meta, weights)
    resB = _run_spmd(ncB, [dict(gs=gs[c], dinv_own=per_core[c]["dinv_own"])
                           for c in range(NC)], NC)

    full = np.zeros(meta["n"], dtype=np.float32)
    for c in range(NC):
        o = np.asarray(resB.results[c]["out"]).reshape(-1)
        org = meta["origin"][c]
        valid = org >= 0
        full[org[valid]] = o[valid]
    return full


# revision 16
# speedup vs baseline: 2.3006x; 1.0374x over previous
"""Self-contained Trainium2 (Bass) kernel for the 2-layer GCN + MLP model.

Strategy (node-parallel, dst-sharded, two SPMD launches):
  * Host prep (index ops only): augment each node's incoming-edge list with a
    self-loop slot, CSR-sort by dst, shard nodes over 8 cores, bucket each
    core's nodes by slot count (deg+1), give every node a fixed (even) number
    of slots (bucket stride).  Slot streams are host-gathered into the slot
    layout as fp8e4m3: xg = x[src] (f-major) and wg = rsqrt(deg[src]+1) via a
    structure-derived palette (zero in padding slots).
  * Launch A (per core): SWDGE DMA casts fp8 -> fp16 during the load;
    y = xg*wg (fp16 tensor_tensor, DVE 2x); in-place halving-tree reduction
    over slots -> agg_f = z1_f/dinv (the self slot completes the aggregate);
    g2_ch = relu(agg@W1*dinv^2 + dinv*b1) = dinv*relu(z1@W1+b1); the layer-2
    input mix W2 commutes with the edge-sum, so A also pre-mixes
    u = g2@W2 (4ch -> 3ch) and stores u as fp8 [P, 3, SUM_M] planes
    (SWDGE cast on the store).
  * Host: gathers u[src] into the same slot layout (index gather only).
  * Launch B (per core): cast-load + halving-tree reduce of the 3-channel u
    slot stream (self slot supplies the own term) -> h2 = sigmoid(agg*dinv+b2)
    directly; then relu(.W3+b3) -> relu(.W4+b4) -> .W5+b5 with weights as
    immediates via tensor_scalar (4x) + tensor_tensor (2x) in fp16 and relu
    as tensor_scalar_max on the DVE (ACT does only sigmoid/sqrt).
  * Host: unpermute per-core outputs back to original node order.

The host only sorts, indexes, pads, concatenates and palette-gathers values
derived from the graph structure; all O(E)/O(N) feature math runs on device.
"""
import numpy as np

import concourse.bass as bass
from concourse.bacc import Bacc
import concourse.mybir as mybir
import concourse.tile as tile

NCORES = 8
N = 1_000_000
P = 128
F32 = mybir.dt.float32
F16 = mybir.dt.float16
F8 = mybir.dt.float8e4
F8NP = mybir.dt.np(mybir.dt.float8e4)
GS_F8 = True
AF = mybir.ActivationFunctionType
OP = mybir.AluOpType


# ----------------------------------------------------------------- host prep
def _choose_strides(max_used):
    max_even = int(max_used) + (int(max_used) & 1)
    ss = [s for s in (2, 4, 6, 8, 10, 12, 14, 16, 18, 20, 22, 24, 26, 28, 32,
                      36, 40, 48, 64, 96, 128, 192, 256, 384, 512)
          if s < max_even]
    ss.append(max_even)
    return ss


def _prep(x, edge_index, ncores=NCORES, n=N):
    npc = n // ncores
    src = np.asarray(edge_index[0]).astype(np.int64)
    dst = np.asarray(edge_index[1]).astype(np.int64)
    x = np.asarray(x, dtype=np.float32)
    deg_in = np.bincount(dst, minlength=n)          # in-degree (no self loop)
    used = deg_in + 1                               # slots incl. self
    strides = _choose_strides(max(int(used.max()), 2))
    strides_arr = np.asarray(strides)
    nb = len(strides)

    # augmented CSR: per dst node, slot 0 = self, slots 1.. = in-edge sources
    order = np.argsort(dst, kind="stable")
    src_s = src[order]
    aug_rowptr = np.zeros(n + 1, dtype=np.int64)
    np.cumsum(used, out=aug_rowptr[1:])
    aug_src = np.empty(n + int(len(src_s)), dtype=np.int64)
    aug_src[aug_rowptr[:-1]] = np.arange(n)
    m = np.ones(len(aug_src), dtype=bool)
    m[aug_rowptr[:-1]] = False
    aug_src[m] = src_s

    # rsqrt palette over deg+1 (structure-derived; fp16)
    pal = (1.0 / np.sqrt(np.arange(1, int(used.max()) + 1, dtype=np.float64)))
    pal = pal.astype(np.float32)
    w_node = pal[deg_in]                            # rsqrt(deg+1) per node

    bucket_of = np.searchsorted(strides_arr, used)

    m_b = np.zeros((ncores, nb), dtype=np.int64)
    node_lists = [[None] * nb for _ in range(ncores)]
    for c in range(ncores):
        lo, hi = c * npc, (c + 1) * npc
        nodes_c = np.arange(lo, hi)
        bk = bucket_of[lo:hi]
        for b in range(nb):
            nl = nodes_c[bk == b]
            node_lists[c][b] = nl
            m_b[c, b] = -(-len(nl) // P)
    m_pad = m_b.max(axis=0)
    SUM_M = -(-int(m_pad.sum()) // 32) * 32
    NPCP = P * SUM_M
    boff = np.concatenate([[0], np.cumsum(m_pad)]).astype(np.int64)
    SLOTS = int((m_pad * P * strides_arr).sum())

    def make_plan(target):
        cp = []
        for b in range(nb):
            s = strides[b]
            if m_pad[b] == 0:
                continue
            mc = max(32, -(-max(1, target // s) // 32) * 32)
            i = 0
            while i < m_pad[b]:
                take = int(min(mc, m_pad[b] - i))
                cp.append((b, s, int(i), take))
                i += take
        return cp
    chunk_plan = make_plan(8192)
    chunk_plan_B = make_plan(8192)

    storage = np.empty(n, dtype=np.int64)
    origin = np.full((ncores, NPCP), -1, dtype=np.int64)
    for c in range(ncores):
        for b in range(nb):
            nl, mb, off = node_lists[c][b], int(m_pad[b]), int(boff[b])
            if len(nl) == 0 or mb == 0:
                continue
            j = np.arange(len(nl))
            p, i = j // mb, j % mb
            sid = p * SUM_M + off + i
            storage[nl] = c * NPCP + sid
            origin[c, sid] = nl

    per_core = []
    for c in range(ncores):
        xg = np.zeros((SLOTS * 2,), dtype=np.float32)
        wgf = np.zeros((SLOTS,), dtype=np.float32)
        idxs = np.full((SLOTS,), ncores * NPCP, dtype=np.int64)  # pad row
        dinv_own = np.ones((NPCP,), dtype=np.float16)
        sbase = 0
        for b in range(nb):
            s, mb = strides[b], int(m_pad[b])
            if mb == 0:
                continue
            nl = node_lists[c][b]
            if len(nl) > 0:
                j = np.arange(len(nl))
                p, i = j // mb, j % mb
                cnt = used[nl]
                node_rep = np.repeat(j, cnt)
                k_in = np.arange(len(node_rep)) - np.repeat(
                    np.concatenate([[0], np.cumsum(cnt)[:-1]]), cnt)
                e_pos = np.repeat(aug_rowptr[nl], cnt) + k_in
                slot = sbase + (p[node_rep] * mb + i[node_rep]) * s + k_in
                sv = aug_src[e_pos]
                slot_fm = sbase * 2 + (p[node_rep] * mb + i[node_rep]) * (2 * s) + k_in
                xg[slot_fm] = x[sv, 0]
                xg[slot_fm + s] = x[sv, 1]
                wgf[slot] = w_node[sv]
                idxs[slot] = storage[sv]
            sbase += P * mb * s
        assert sbase == SLOTS

        valid = origin[c] >= 0
        ov = origin[c][valid]
        dinv_own[valid] = w_node[ov].astype(np.float16)
        per_core.append(dict(xg=xg.astype(F8NP), wg=wgf.astype(F8NP),
                             idxs=idxs, dinv_own=dinv_own))

    meta = dict(strides=strides, m_pad=m_pad, SUM_M=SUM_M, NPCP=NPCP,
                boff=boff, SLOTS=SLOTS, chunk_plan=chunk_plan,
                chunk_plan_B=chunk_plan_B, origin=origin,
                ncores=ncores, n=n)
    return per_core, meta


# ------------------------------------------------------------- device common
def _sbases(meta):
    sbases = {}
    sb = 0
    for b, s in enumerate(meta["strides"]):
        sbases[b] = sb
        sb += P * int(meta["m_pad"][b]) * s
    return sbases


def _tree_reduce(nc, t4, s, final_out):
    """In-place halving-tree sum over the last axis of t4 [P, mc, nch, s]
    (s even, >= 2); the final level writes per-channel totals to
    final_out [P, mc, nch] in a single op."""
    cur = s
    while cur > 2:
        h = cur // 2
        nc.vector.tensor_tensor(out=t4[:, :, :, 0:h], in0=t4[:, :, :, 0:h],
                                in1=t4[:, :, :, cur - h:cur], op=OP.add)
        cur -= h
    nc.vector.tensor_tensor(out=final_out, in0=t4[:, :, :, 0],
                            in1=t4[:, :, :, 1], op=OP.add)


def _dinv_chain(nc, res, dinv_own, SUM_M):
    dinvb = res.tile([P, SUM_M], F16, tag="dinvb")
    nc.sync.dma_start(out=dinvb[:],
                      in_=dinv_own[:].rearrange("(p j) -> p j", p=P))
    return dinvb


# --------------------------------------------------------- device build: A
def _build_A(meta, W1b, W2, reps=1):
    SUM_M, SLOTS = meta["SUM_M"], meta["SLOTS"]
    m_pad, boff = meta["m_pad"], meta["boff"]
    plan = meta["chunk_plan"]
    sbases = _sbases(meta)

    nc = Bacc(num_devices=meta["ncores"])
    xg = nc.declare_dram_parameter("xg", [SLOTS * 2], F8, isOutput=False)
    wg = nc.declare_dram_parameter("wg", [SLOTS], F8, isOutput=False)
    dinv_own = nc.declare_dram_parameter("dinv_own", [meta["NPCP"]], F16,
                                         isOutput=False)
    g2out = nc.declare_dram_parameter("g2out", [P, 3, SUM_M], F8,
                                      isOutput=True)

    with tile.TileContext(nc) as tc:
        with tc.tile_pool(name="res", bufs=1) as res:
            for _ in range(reps):
                dinvb = _dinv_chain(nc, res, dinv_own, SUM_M)
                dsqb = res.tile([P, SUM_M], F16, tag="dsqb")
                nc.vector.tensor_tensor(out=dsqb[:], in0=dinvb[:],
                                        in1=dinvb[:], op=OP.mult)

                agg2t = res.tile([P, SUM_M, 2], F16, tag="agg2t")
                nc.vector.memset(agg2t[:], 0.0)

                with tc.tile_pool(name="l1", bufs=3) as st:
                    for (b, s, i0, mc) in plan:
                        mb = int(m_pad[b])
                        xv = xg[2 * sbases[b]:2 * (sbases[b] + P * mb * s)] \
                            .rearrange("(p i fk) -> p i fk", p=P, i=mb)[:, i0:i0 + mc, :]
                        wv = wg[sbases[b]:sbases[b] + P * mb * s] \
                            .rearrange("(p i k) -> p i k", p=P, i=mb)[:, i0:i0 + mc, :]
                        xt = st.tile([P, mc, 2 * s], F16, tag="xg")
                        wt = st.tile([P, mc, s], F16, tag="wg")
                        nc.gpsimd.dma_start(out=xt[:], in_=xv)
                        nc.gpsimd.dma_start(out=wt[:], in_=wv)
                        for f in range(2):
                            sl = xt[:, :, f * s:(f + 1) * s]
                            nc.vector.tensor_tensor(out=sl, in0=sl, in1=wt[:],
                                                    op=OP.mult)
                        j0 = int(boff[b]) + i0
                        xt4 = xt[:].rearrange("p i (f k) -> p i f k", k=s)
                        _tree_reduce(nc, xt4, s, agg2t[:, j0:j0 + mc, :])

                # g2_ch = relu(agg@W1*dinv^2 + dinv*b1); u_o = sum_ch g2_ch*W2[ch,o]
                af3 = [res.tile([P, SUM_M], F16, tag=f"af3{f}",
                                name=f"af3{f}") for f in range(2)]
                for f in range(2):
                    nc.vector.tensor_tensor(out=af3[f][:], in0=agg2t[:, :, f],
                                            in1=dsqb[:], op=OP.mult)
                g2r = [res.tile([P, SUM_M], F16, tag=f"g2r{o}",
                                name=f"g2r{o}") for o in range(4)]
                ti = res.tile([P, SUM_M], F16, tag="ti")
                for o in range(4):
                    go = g2r[o]
                    nc.vector.tensor_scalar(out=go[:], in0=af3[0][:],
                                            scalar1=float(W1b[0, o]),
                                            scalar2=None, op0=OP.mult)
                    nc.vector.tensor_scalar(out=ti[:], in0=af3[1][:],
                                            scalar1=float(W1b[1, o]),
                                            scalar2=None, op0=OP.mult)
                    nc.vector.tensor_tensor(out=go[:], in0=go[:], in1=ti[:],
                                            op=OP.add)
                    nc.vector.tensor_scalar(out=ti[:], in0=dinvb[:],
                                            scalar1=float(W1b[2, o]),
                                            scalar2=None, op0=OP.mult)
                    nc.vector.tensor_tensor(out=go[:], in0=go[:], in1=ti[:],
                                            op=OP.add)
                    nc.vector.tensor_scalar_max(out=go[:], in0=go[:],
                                                scalar1=0.0)
                up = res.tile([P, 3, SUM_M], F16, tag="up")
                for o in range(3):
                    uo = up[:, o, :]
                    nc.vector.tensor_scalar(out=uo, in0=g2r[0][:],
                                            scalar1=float(W2[0, o]),
                                            scalar2=None, op0=OP.mult)
                    for ch in range(1, 4):
                        nc.vector.tensor_scalar(out=ti[:], in0=g2r[ch][:],
                                                scalar1=float(W2[ch, o]),
                                                scalar2=None, op0=OP.mult)
                        nc.vector.tensor_tensor(out=uo, in0=uo, in1=ti[:],
                                                op=OP.add)
                nc.gpsimd.dma_start(out=g2out[:], in_=up[:])
    return nc


# --------------------------------------------------------- device build: B
def _build_B(meta, weights, reps=1):
    SUM_M, SLOTS = meta["SUM_M"], meta["SLOTS"]
    m_pad, boff = meta["m_pad"], meta["boff"]
    plan = meta["chunk_plan_B"]
    sbases = _sbases(meta)
    W2, b2 = weights["W2"], weights["b2"]
    W3, b3 = weights["W3"], weights["b3"]
    W4, b4 = weights["W4"], weights["b4"]
    W5, b5 = weights["W5"], weights["b5"]

    nc = Bacc(num_devices=meta["ncores"])
    gs = nc.declare_dram_parameter("gs", [SLOTS * 3],
                                   F8 if GS_F8 else F16, isOutput=False)
    dinv_own = nc.declare_dram_parameter("dinv_own", [meta["NPCP"]], F16,
                                         isOutput=False)
    out = nc.declare_dram_parameter("out", [P, SUM_M], F32, isOutput=True)

    with tile.TileContext(nc) as tc:
        with tc.tile_pool(name="res", bufs=1) as res:
            for _ in range(reps):
                dinvb = _dinv_chain(nc, res, dinv_own, SUM_M)

                agg3 = res.tile([P, SUM_M, 3], F16, tag="agg3")
                nc.vector.memset(agg3[:], 0.0)

                with tc.tile_pool(name="l2", bufs=3) as st:
                    for (b, s, i0, mc) in plan:
                        mb = int(m_pad[b])
                        gv = gs[3 * sbases[b]:3 * (sbases[b] + P * mb * s)] \
                            .rearrange("(p i fk) -> p i fk", p=P, i=mb)[:, i0:i0 + mc, :]
                        gt = st.tile([P, mc, 3 * s], F16, tag="gath")
                        if GS_F8:
                            nc.gpsimd.dma_start(out=gt[:], in_=gv)
                        else:
                            nc.sync.dma_start(out=gt[:], in_=gv)
                        j0 = int(boff[b]) + i0
                        gt4 = gt[:].rearrange("p i (f k) -> p i f k", k=s)
                        _tree_reduce(nc, gt4, s, agg3[:, j0:j0 + mc, :])

                # h2_o = sigmoid(aggu_o*dinv + b2_o)   (W2 pre-mixed in launch A)
                h2 = []
                for o in range(3):
                    h = res.tile([P, SUM_M], F16, tag=f"h2_{o}",
                                 name=f"h2_{o}")
                    bt = res.tile([P, 1], F32, tag=f"h2b{o}", name=f"h2b{o}")
                    nc.vector.memset(bt[:], float(b2[o]))
                    nc.vector.tensor_tensor(out=h[:], in0=agg3[:, :, o],
                                            in1=dinvb[:], op=OP.mult)
                    nc.scalar.activation(out=h[:], in_=h[:], func=AF.Sigmoid,
                                         bias=bt[:])
                    h2.append(h)

                ti = res.tile([P, SUM_M], F16, tag="ti")

                def dense(ins_, Wm, bias, act, tagp, och):
                    outs_ = []
                    for o in range(och):
                        acc = res.tile([P, SUM_M], F16, tag=f"{tagp}{o}",
                                       name=f"{tagp}{o}")
                        nc.vector.tensor_scalar(out=acc[:], in0=ins_[0][:],
                                                scalar1=float(Wm[0, o]),
                                                scalar2=float(bias[o]),
                                                op0=OP.mult, op1=OP.add)
                        for i in range(1, len(ins_)):
                            nc.vector.tensor_scalar(out=ti[:], in0=ins_[i][:],
                                                    scalar1=float(Wm[i, o]),
                                                    scalar2=None, op0=OP.mult)
                            nc.vector.tensor_tensor(out=acc[:], in0=acc[:],
                                                    in1=ti[:], op=OP.add)
                        if act == "sigmoid":
                            nc.scalar.activation(out=acc[:], in_=acc[:],
                                                 func=AF.Sigmoid)
                        elif act == "relu":
                            nc.vector.tensor_scalar_max(out=acc[:], in0=acc[:],
                                                        scalar1=0.0)
                        outs_.append(acc)
                    return outs_

                h = dense(h2, W3, b3, "relu", "h3_", 4)
                h = dense(h, W4, b4, "relu", "h4_", 3)
                h5 = dense(h, W5, b5, None, "h5_", 1)
                nc.gpsimd.dma_start(out=out[:], in_=h5[0][:])
    return nc


# ------------------------------------------------------------------ driver
def _run_spmd(nc, in_maps, ncores):
    from concourse.bass_utils import run_bass_kernel_spmd
    if not nc.is_finalized():
        nc.finalize()
    return run_bass_kernel_spmd(nc, in_maps, core_ids=list(range(ncores)))


def host_gather_g2(meta, per_core, g2_slices):
    """g2_slices[c]: [P, 3, SUM_M] fp8 u-planes from launch A. Returns
    per-core slot streams [SLOTS*3] in ch-major slot layout (index gather)."""
    NC, NPCP = meta["ncores"], meta["NPCP"]
    strides, m_pad = meta["strides"], meta["m_pad"]
    g2_nodes = [np.asarray(g2_slices[c]).transpose(0, 2, 1).reshape(NPCP, 3)
                for c in range(NC)]
    g2_full = np.concatenate(
        g2_nodes + [np.zeros((1, 3), g2_nodes[0].dtype)], axis=0)
    out = []
    for c in range(NC):
        g = g2_full[per_core[c]["idxs"]]          # [SLOTS, 3] slot-major
        fm = np.empty((meta["SLOTS"] * 3,), dtype=g2_full.dtype)
        sbase = 0
        for b, s in enumerate(strides):
            mb = int(m_pad[b])
            if mb == 0:
                continue
            nseg = P * mb * s
            seg = g[sbase:sbase + nseg].reshape(P * mb, s, 3)
            fm[sbase * 3:(sbase + nseg) * 3] = \
                seg.transpose(0, 2, 1).reshape(-1)
            sbase += nseg
        out.append(fm if GS_F8 else fm.astype(np.float16))
    return out


def kernel(x, edge_index, W1, b1, W2, b2, W3, b3, W4, b4, W5, b5):
    x = np.asarray(x, dtype=np.float32)
    per_core, meta = _prep(x, edge_index)
    W1b = np.concatenate([np.asarray(W1), np.asarray(b1)[None, :]], axis=0)
    weights = dict(W2=np.asarray(W2), b2=np.asarray(b2),
                   W3=np.asarray(W3), b3=np.asarray(b3),
                   W4=np.asarray(W4), b4=np.asarray(b4),
                   W5=np.asarray(W5), b5=np.asarray(b5))
    NC = meta["ncores"]

    ncA = _build_A(meta, W1b, np.asarray(W2))
    resA = _run_spmd(ncA, [{k: d[k] for k in ("xg", "wg", "dinv_own")}
                           for d in per_core], NC)
    g2_slices = [resA.results[c]["g2out"] for c in range(NC)]

    gs = host_gather_g2(meta, per_core, g2_slices)
    ncB = _build_B(meta, weights)
    resB = _run_spmd(ncB, [dict(gs=gs[c], dinv_own=per_core[c]["dinv_own"])
                           for c in range(NC)], NC)

    full = np.zeros(meta["n"], dtype=np.float32)
    for c in range(NC):
        o = np.asarray(resB.results[c]["out"]).reshape(-1)
        org = meta["origin"][c]
        valid = org >= 0
        full[org[valid]] = o[valid]
    return full
